# revision 1
# baseline (speedup 1.0000x reference)
"""Class-balanced softmax cross-entropy loss on 8 Trainium2 NeuronCores.

Math (per the reference nn.Module):
  counts N_c   = histogram of target over classes (whole batch)
  weights w_c  = (1-beta)/(1-beta^N_c), 0 where N_c == 0
  logp         = log_softmax(logits, axis=C)
  loss         = -sum_pix w[t] * logp[t_pix] / sum_pix w[t]

Decomposition used here: per core (data-parallel over batch B=8, one batch
item per core) compute per-class partials
  N_c = sum(target == c)
  A_c = sum_{target==c} logits[c]
  B_c = sum_{target==c} lse          (lse = log(sum_c' exp(logits[c'])))
Then on host: N = sum_cores N_c; w from N;
  loss = -(w . (A - B)) / (w . N)
No collectives needed; each core returns 3*19 floats.
"""

import numpy as np
from contextlib import ExitStack
import sys

for _p in ("/opt/trn_rl_repo",):
    if _p not in sys.path:
        sys.path.insert(0, _p)

from concourse import bass, mybir, tile
from concourse.bass_utils import run_bass_kernel_spmd

NCLASS = 19
BETA = 0.999
NCORES = 8
HW = 512 * 1024          # pixels per batch item (= per core)
P = 128                  # SBUF partitions
COLS = HW // P           # 4096
F = 512                  # free-dim chunk
NCHUNK = COLS // F       # 8

f32 = mybir.dt.float32
i32 = mybir.dt.int32
AF = mybir.ActivationFunctionType
ALU = mybir.AluOpType

# accumulator column layout: [A (NCHUNK*NCLASS) | B (...) | N (...)]
SEC = NCHUNK * NCLASS          # 152
ACC_COLS = 3 * SEC             # 456


def _build():
    """Raw-bass pipeline with manual semaphores.

    Engine roles per chunk k (buffer half h=k%2):
      ACT: issue X/T DMAs, exp x19, log; DVE: reduce(sumexp), 57 stt/ts
      accumulations; PE: final partition-reduce matmul.
    Cross-engine edges via explicit wait_ge/then_inc; within-engine order is
    program order. Transitive implications (exp done => X landed) are used
    to keep the wait count low.
    """
    nc = bass.Bass()
    logits = nc.declare_dram_parameter("logits", [NCLASS, P, COLS], f32, isOutput=False)
    target = nc.declare_dram_parameter("target", [P, COLS], i32, isOutput=False)
    out = nc.declare_dram_parameter("out", [1, ACC_COLS], f32, isOutput=True)

    EF = NCLASS * F
    X2 = nc.alloc_sbuf_tensor("X2", [P, 2 * EF], f32)
    E2 = nc.alloc_sbuf_tensor("E2", [P, 2 * EF], f32)
    Ti2 = nc.alloc_sbuf_tensor("Ti2", [P, 2 * F], i32)
    Tf2 = nc.alloc_sbuf_tensor("Tf2", [P, 2 * F], f32)
    S2 = nc.alloc_sbuf_tensor("S2", [P, 2 * F], f32)
    L2 = nc.alloc_sbuf_tensor("L2", [P, 2 * F], f32)
    junk = nc.alloc_sbuf_tensor("junk", [P, F], f32)
    ABN = nc.alloc_sbuf_tensor("ABN", [P, ACC_COLS], f32)
    ones = nc.alloc_sbuf_tensor("ones", [P, 1], f32)
    ones_f = nc.alloc_sbuf_tensor("ones_f", [P, F], f32)
    res = nc.alloc_sbuf_tensor("res", [1, ACC_COLS], f32)
    ps = nc.alloc_psum_tensor("ps", [1, ACC_COLS], f32)

    with (
        nc.Block() as block,
        nc.semaphore("sem_x") as sem_x,
        nc.semaphore("sem_t") as sem_t,
        nc.semaphore("sem_exp") as sem_exp,
        nc.semaphore("sem_red") as sem_red,
        nc.semaphore("sem_log") as sem_log,
        nc.semaphore("sem_done") as sem_done,
        nc.semaphore("sem_mm") as sem_mm,
        nc.semaphore("sem_out") as sem_out,
    ):
        @block.scalar
        def _(act):
            for k in range(NCHUNK):
                h = k % 2
                if k >= 2:
                    act.wait_ge(sem_done, k - 1)   # bufs of chunk k-2 free
                act.dma_start(
                    X2[:, h * EF:(h + 1) * EF].rearrange("p (c f) -> p c f", c=NCLASS),
                    logits[:, :, k * F:(k + 1) * F].rearrange("c p f -> p c f"),
                ).then_inc(sem_x, 16)
                act.dma_start(
                    Ti2[:, h * F:(h + 1) * F], target[:, k * F:(k + 1) * F],
                ).then_inc(sem_t, 16)
                act.wait_ge(sem_x, 16 * (k + 1))
                for c in range(NCLASS):
                    ins = act.activation(
                        E2[:, h * EF + c * F: h * EF + (c + 1) * F],
                        X2[:, h * EF + c * F: h * EF + (c + 1) * F], AF.Exp)
                    if c == NCLASS - 1:
                        ins.then_inc(sem_exp, 1)
                act.wait_ge(sem_red, k + 1)
                act.activation(
                    L2[:, h * F:(h + 1) * F], S2[:, h * F:(h + 1) * F], AF.Ln,
                ).then_inc(sem_log, 1)
            # tail: psum -> sbuf -> dram
            act.wait_ge(sem_mm, 1)
            act.copy(res[:], ps[:])
            act.dma_start(out[:, :], res[:]).then_inc(sem_out, 16)
            act.wait_ge(sem_out, 16)

        @block.vector
        def _(dve):
            dve.memset(ABN[:], 0.0)
            dve.memset(ones[:], 1.0)
            dve.memset(ones_f[:], 1.0)
            for k in range(NCHUNK):
                h = k % 2
                dve.wait_ge(sem_exp, k + 1)   # E ready (implies X landed)
                dve.tensor_reduce(
                    S2[:, h * F:(h + 1) * F],
                    E2[:, h * EF:(h + 1) * EF].rearrange("p (c f) -> p f c", c=NCLASS),
                    axis=mybir.AxisListType.X, op=ALU.add,
                ).then_inc(sem_red, 1)
                dve.wait_ge(sem_t, 16 * (k + 1))
                Ti = Tf2[:, h * F:(h + 1) * F]
                dve.tensor_copy(Ti[:], Ti2[:, h * F:(h + 1) * F])
                for c in range(NCLASS):
                    dve.scalar_tensor_tensor(
                        out=junk[:], in0=Ti[:], scalar=float(c),
                        in1=X2[:, h * EF + c * F: h * EF + (c + 1) * F],
                        op0=ALU.is_equal, op1=ALU.mult,
                        accum_out=ABN[:, 0 * SEC + k * NCLASS + c: 0 * SEC + k * NCLASS + c + 1])
                dve.wait_ge(sem_log, k + 1)
                LSE = L2[:, h * F:(h + 1) * F]
                for c in range(NCLASS):
                    dve.scalar_tensor_tensor(
                        out=junk[:], in0=Ti[:], scalar=float(c), in1=LSE[:],
                        op0=ALU.is_equal, op1=ALU.mult,
                        accum_out=ABN[:, 1 * SEC + k * NCLASS + c: 1 * SEC + k * NCLASS + c + 1])
                for c in range(NCLASS):
                    # counts: single-src tensor_scalar runs in 2x_2P mode;
                    # op1 is the accum reduce op (add)
                    ins = dve.tensor_scalar(
                        out=junk[:], in0=Ti[:], scalar1=float(c), scalar2=None,
                        op0=ALU.is_equal, op1=ALU.add,
                        accum_out=ABN[:, 2 * SEC + k * NCLASS + c: 2 * SEC + k * NCLASS + c + 1])
                    if c == NCLASS - 1:
                        ins.then_inc(sem_done, 1)

        @block.tensor
        def _(pe):
            pe.wait_ge(sem_done, NCHUNK)
            pe.matmul(ps[:], lhsT=ones[:], rhs=ABN[:], start=True, stop=True).then_inc(sem_mm, 1)

    return nc


def _build_tile_unused():
    nc = bass.Bass()
    logits = nc.declare_dram_parameter("logits", [NCLASS, P, COLS], f32, isOutput=False)
    target = nc.declare_dram_parameter("target", [P, COLS], i32, isOutput=False)
    out = nc.declare_dram_parameter("out", [1, ACC_COLS], f32, isOutput=True)

    with ExitStack() as ctx:
        tc = ctx.enter_context(tile.TileContext(nc))
        xpool = ctx.enter_context(tc.tile_pool(name="x", bufs=2))
        tpool = ctx.enter_context(tc.tile_pool(name="t", bufs=2))
        accpool = ctx.enter_context(tc.tile_pool(name="acc", bufs=1))
        pspool = ctx.enter_context(tc.tile_pool(name="ps", bufs=1, space="PSUM"))

        EF = NCLASS * F
        ABN = accpool.tile([P, ACC_COLS], f32)
        nc.vector.memset(ABN[:], 0.0)
        ones = accpool.tile([P, 1], f32)
        nc.vector.memset(ones[:], 1.0)
        # persistent manually double-buffered scratch (avoids Tile pool
        # release-waits, which overflow the 1-sync-wait ISA limit)
        Ebuf = accpool.tile([P, 2 * EF], f32)
        Sbuf = accpool.tile([P, 2 * F], f32)
        Lbuf = accpool.tile([P, 2 * F], f32)
        junk = accpool.tile([P, F], f32)
        pabs = accpool.tile([P, 1], f32)   # DVE absorber dst
        pdve = accpool.tile([P, 1], f32)   # DVE->ACT probe src
        pscr = accpool.tile([P, 1], f32)   # ACT probe dst

        probes = {}
        for k in range(NCHUNK):
            h = k % 2
            X = xpool.tile([P, EF], f32, tag="x")
            xdma = nc.scalar.dma_start(
                X[:].rearrange("p (c f) -> p c f", c=NCLASS),
                logits[:, :, k * F:(k + 1) * F].rearrange("c p f -> p c f"))
            Ti = tpool.tile([P, F], i32, tag="ti")
            tdma = nc.scalar.dma_start(Ti[:], target[:, k * F:(k + 1) * F])
            if k >= 2:
                # Order this chunk's DMAs after the probe that made ACT
                # observe DVE's consumption of the recycled buffers, so the
                # DMACopy needs no extra sync-wait (1-wait ISA limit).
                tile.add_dep_helper(xdma.ins, probes[k - 2], reason="recycle absorb")
                tile.add_dep_helper(tdma.ins, probes[k - 2], reason="recycle absorb")

            E = Ebuf[:, h * EF:(h + 1) * EF]
            for c in range(NCLASS):
                nc.scalar.activation(E[:, c * F:(c + 1) * F], X[:, c * F:(c + 1) * F], AF.Exp)

            S = Sbuf[:, h * F:(h + 1) * F]
            nc.vector.tensor_reduce(
                S[:], E[:].rearrange("p (c f) -> p f c", c=NCLASS),
                axis=mybir.AxisListType.X, op=ALU.add)
            LSE = Lbuf[:, h * F:(h + 1) * F]
            log_ins = nc.scalar.activation(LSE[:], S[:], AF.Ln).ins

            # Drain instructions accept many sync-waits; use one as the
            # absorber for ALL of this chunk's cross-engine edges so every
            # following DVE instruction needs at most its self-wait.
            dr = nc.vector.drain()
            tile.add_dep_helper(dr.ins, xdma.ins, reason="absorb x dma")
            tile.add_dep_helper(dr.ins, tdma.ins, reason="absorb t dma")
            tile.add_dep_helper(dr.ins, log_ins, reason="absorb log")
            for c in range(NCLASS):
                # A_c partial: sum over free of (T==c)*logits_c
                stt = nc.vector.scalar_tensor_tensor(
                    out=junk[:], in0=Ti[:], scalar=float(c), in1=X[:, c * F:(c + 1) * F],
                    op0=ALU.is_equal, op1=ALU.mult,
                    accum_out=ABN[:, 0 * SEC + k * NCLASS + c: 0 * SEC + k * NCLASS + c + 1])
                if c == 0:
                    # force the drain ahead of the whole stt block (ordered
                    # among themselves by the junk WAW chain)
                    tile.add_dep_helper(stt.ins, dr.ins, reason="stt after drain")
            for c in range(NCLASS):
                # B_c partial: sum over free of (T==c)*lse
                nc.vector.scalar_tensor_tensor(
                    out=junk[:], in0=Ti[:], scalar=float(c), in1=LSE[:],
                    op0=ALU.is_equal, op1=ALU.mult,
                    accum_out=ABN[:, 1 * SEC + k * NCLASS + c: 1 * SEC + k * NCLASS + c + 1])
            for c in range(NCLASS):
                # N_c partial: sum over free of (T==c)
                nc.vector.tensor_scalar(
                    out=junk[:], in0=Ti[:], scalar1=float(c), scalar2=1.0,
                    op0=ALU.is_equal, op1=ALU.mult,
                    accum_out=ABN[:, 2 * SEC + k * NCLASS + c: 2 * SEC + k * NCLASS + c + 1])
            nc.vector.tensor_copy(pdve[:], junk[:, 0:1])
            probes[k] = nc.scalar.copy(pscr[:], pdve[:]).ins

        ps = pspool.tile([1, ACC_COLS], f32)
        mm = nc.tensor.matmul(ps[:], lhsT=ones[:], rhs=ABN[:], start=True, stop=True)
        dr2 = nc.scalar.drain()
        tile.add_dep_helper(dr2.ins, mm.ins, reason="absorb matmul")
        res = accpool.tile([1, ACC_COLS], f32)
        nc.scalar.copy(res[:], ps[:])
        nc.scalar.dma_start(out[:, :], res[:])

    return nc


_CACHE = {}


def _get_nc():
    if "nc" not in _CACHE:
        _CACHE["nc"] = _build()
    return _CACHE["nc"]


def _run(logits, target, trace=False):
    nc = _get_nc()
    in_maps = []
    for i in range(NCORES):
        in_maps.append({
            "logits": np.ascontiguousarray(logits[i].reshape(NCLASS, P, COLS)),
            "target": np.ascontiguousarray(target[i].reshape(P, COLS)),
        })
    r = run_bass_kernel_spmd(nc, in_maps, core_ids=list(range(NCORES)), trace=trace)
    return r


def _combine(results):
    A = np.zeros(NCLASS, np.float64)
    B = np.zeros(NCLASS, np.float64)
    N = np.zeros(NCLASS, np.float64)
    for i in range(NCORES):
        r = results[i]["out"].astype(np.float64).reshape(3, NCHUNK, NCLASS).sum(axis=1)
        A += r[0]
        B += r[1]
        N += r[2]
    w = np.where(N > 0, (1.0 - BETA) / (1.0 - BETA ** N), 0.0)
    num = float((w * (A - B)).sum())
    den = float((w * N).sum())
    return np.float32(-num / den)


def kernel(logits, target):
    assert logits.shape == (NCORES, NCLASS, 512, 1024) and logits.dtype == np.float32
    assert target.shape == (NCORES, 512, 1024) and target.dtype == np.int32
    r = _run(logits, target, trace=False)
    return _combine(r.results)



# revision 4
# speedup vs baseline: 1.2549x; 1.2549x over previous
"""Class-balanced softmax cross-entropy loss on 8 Trainium2 NeuronCores.

Math (per the reference nn.Module):
  counts N_c   = histogram of target over classes (whole batch)
  weights w_c  = (1-beta)/(1-beta^N_c), 0 where N_c == 0
  logp         = log_softmax(logits, axis=C)
  loss         = -sum_pix w[t] * logp[t_pix] / sum_pix w[t]

Decomposition used here: per core (data-parallel over batch B=8, one batch
item per core) compute per-class partials
  N_c = sum(target == c)
  A_c = sum_{target==c} logits[c]
  B_c = sum_{target==c} lse          (lse = log(sum_c' exp(logits[c'])))
Then on host: N = sum_cores N_c; w from N;
  loss = -(w . (A - B)) / (w . N)
No collectives needed; each core returns 3*19 floats.
"""

import numpy as np
from contextlib import ExitStack
import sys

for _p in ("/opt/trn_rl_repo",):
    if _p not in sys.path:
        sys.path.insert(0, _p)

from concourse import bass, mybir, tile
from concourse.bass_utils import run_bass_kernel_spmd

NCLASS = 19
BETA = 0.999
NCORES = 8
HW = 512 * 1024          # pixels per batch item (= per core)
P = 128                  # SBUF partitions
COLS = HW // P           # 4096
F = 512                  # free-dim chunk
NCHUNK = COLS // F       # 8

f32 = mybir.dt.float32
i32 = mybir.dt.int32
AF = mybir.ActivationFunctionType
ALU = mybir.AluOpType

# accumulator column layout: [A (NCHUNK*NCLASS) | B (...) | N (...)]
SEC = NCHUNK * NCLASS          # 152
ACC_COLS = 3 * SEC             # 456


def _build():
    """Raw-bass pipeline with manual semaphores.

    Engine roles per chunk k (buffer half h=k%2):
      ACT: issue X/T DMAs, exp x19, log; DVE: reduce(sumexp), 57 stt/ts
      accumulations; PE: final partition-reduce matmul.
    Cross-engine edges via explicit wait_ge/then_inc; within-engine order is
    program order. Transitive implications (exp done => X landed) are used
    to keep the wait count low.
    """
    nc = bass.Bass()
    logits = nc.declare_dram_parameter("logits", [NCLASS, P, COLS], f32, isOutput=False)
    target = nc.declare_dram_parameter("target", [P, COLS], i32, isOutput=False)
    out = nc.declare_dram_parameter("out", [1, ACC_COLS], f32, isOutput=True)

    EF = NCLASS * F
    bf16 = mybir.dt.bfloat16
    X2 = nc.alloc_sbuf_tensor("X2", [P, 2 * EF], f32)
    E2 = nc.alloc_sbuf_tensor("E2", [P, 2 * EF], bf16)
    Ti2 = nc.alloc_sbuf_tensor("Ti2", [P, 2 * F], i32)
    Tf2 = nc.alloc_sbuf_tensor("Tf2", [P, 2 * F], f32)
    S2 = nc.alloc_sbuf_tensor("S2", [P, 2 * F], f32)
    L2 = nc.alloc_sbuf_tensor("L2", [P, 2 * F], f32)
    junk = nc.alloc_sbuf_tensor("junk", [P, F], f32)
    ABN = nc.alloc_sbuf_tensor("ABN", [P, ACC_COLS], f32)
    ones = nc.alloc_sbuf_tensor("ones", [P, 1], f32)
    ones_f = nc.alloc_sbuf_tensor("ones_f", [P, F], f32)
    res = nc.alloc_sbuf_tensor("res", [1, ACC_COLS], f32)
    ps = nc.alloc_psum_tensor("ps", [1, ACC_COLS], f32)

    with (
        nc.Block() as block,
        nc.semaphore("sem_x") as sem_x,
        nc.semaphore("sem_t") as sem_t,
        nc.semaphore("sem_exp") as sem_exp,
        nc.semaphore("sem_red") as sem_red,
        nc.semaphore("sem_log") as sem_log,
        nc.semaphore("sem_done") as sem_done,
        nc.semaphore("sem_mm") as sem_mm,
        nc.semaphore("sem_out") as sem_out,
    ):
        @block.scalar
        def _(act):
            for k in range(NCHUNK):
                h = k % 2
                if k >= 2:
                    act.wait_ge(sem_done, k - 1)   # bufs of chunk k-2 free
                act.dma_start(
                    X2[:, h * EF:(h + 1) * EF].rearrange("p (c f) -> p c f", c=NCLASS),
                    logits[:, :, k * F:(k + 1) * F].rearrange("c p f -> p c f"),
                ).then_inc(sem_x, 16)
                act.dma_start(
                    Ti2[:, h * F:(h + 1) * F], target[:, k * F:(k + 1) * F],
                ).then_inc(sem_t, 16)
                act.wait_ge(sem_x, 16 * (k + 1))
                act.activation(
                    E2[:, h * EF:(h + 1) * EF],
                    X2[:, h * EF:(h + 1) * EF], AF.Exp,
                ).then_inc(sem_exp, 1)
                act.wait_ge(sem_red, k + 1)
                act.activation(
                    L2[:, h * F:(h + 1) * F], E2[:, h * EF: h * EF + F], AF.Ln,
                ).then_inc(sem_log, 1)
            # tail: psum -> sbuf -> dram
            act.wait_ge(sem_mm, 1)
            act.copy(res[:], ps[:])
            act.dma_start(out[:, :], res[:]).then_inc(sem_out, 16)
            act.wait_ge(sem_out, 16)

        @block.vector
        def _(dve):
            dve.memset(ABN[:], 0.0)
            dve.memset(ones[:], 1.0)
            dve.memset(ones_f[:], 1.0)
            for k in range(NCHUNK):
                h = k % 2
                dve.wait_ge(sem_exp, k + 1)   # E ready (implies X landed)
                # sumexp via in-place bf16 fold tree (tt-add runs 2x in bf16;
                # the strided tensor_reduce it replaces ran at ~1.7ns/elem)
                E = E2[:, h * EF:(h + 1) * EF]
                dve.tensor_tensor(out=E[:, 0:9 * F], in0=E[:, 0:9 * F],
                                  in1=E[:, 9 * F:18 * F], op=ALU.add)
                dve.tensor_tensor(out=E[:, 0:4 * F], in0=E[:, 0:4 * F],
                                  in1=E[:, 4 * F:8 * F], op=ALU.add)
                dve.tensor_tensor(out=E[:, 0:2 * F], in0=E[:, 0:2 * F],
                                  in1=E[:, 2 * F:4 * F], op=ALU.add)
                dve.tensor_tensor(out=E[:, 0:F], in0=E[:, 0:F],
                                  in1=E[:, F:2 * F], op=ALU.add)
                dve.tensor_tensor(out=E[:, 0:F], in0=E[:, 0:F],
                                  in1=E[:, 8 * F:9 * F], op=ALU.add)
                dve.tensor_tensor(
                    out=E[:, 0:F], in0=E[:, 0:F],
                    in1=E[:, 18 * F:19 * F], op=ALU.add,
                ).then_inc(sem_red, 1)
                dve.wait_ge(sem_t, 16 * (k + 1))
                Ti = Tf2[:, h * F:(h + 1) * F]
                dve.tensor_copy(Ti[:], Ti2[:, h * F:(h + 1) * F])
                for c in range(NCLASS):
                    dve.scalar_tensor_tensor(
                        out=junk[:], in0=Ti[:], scalar=float(c),
                        in1=X2[:, h * EF + c * F: h * EF + (c + 1) * F],
                        op0=ALU.is_equal, op1=ALU.mult,
                        accum_out=ABN[:, 0 * SEC + k * NCLASS + c: 0 * SEC + k * NCLASS + c + 1])
                dve.wait_ge(sem_log, k + 1)
                LSE = L2[:, h * F:(h + 1) * F]
                for c in range(NCLASS):
                    dve.scalar_tensor_tensor(
                        out=junk[:], in0=Ti[:], scalar=float(c), in1=LSE[:],
                        op0=ALU.is_equal, op1=ALU.mult,
                        accum_out=ABN[:, 1 * SEC + k * NCLASS + c: 1 * SEC + k * NCLASS + c + 1])
                for c in range(NCLASS):
                    # counts: single-src tensor_scalar runs in 2x_2P mode;
                    # op1 is the accum reduce op (add)
                    ins = dve.tensor_scalar(
                        out=junk[:], in0=Ti[:], scalar1=float(c), scalar2=None,
                        op0=ALU.is_equal, op1=ALU.add,
                        accum_out=ABN[:, 2 * SEC + k * NCLASS + c: 2 * SEC + k * NCLASS + c + 1])
                    if c == NCLASS - 1:
                        ins.then_inc(sem_done, 1)

        @block.tensor
        def _(pe):
            pe.wait_ge(sem_done, NCHUNK)
            pe.matmul(ps[:], lhsT=ones[:], rhs=ABN[:], start=True, stop=True).then_inc(sem_mm, 1)

    return nc


def _build_tile_unused():
    nc = bass.Bass()
    logits = nc.declare_dram_parameter("logits", [NCLASS, P, COLS], f32, isOutput=False)
    target = nc.declare_dram_parameter("target", [P, COLS], i32, isOutput=False)
    out = nc.declare_dram_parameter("out", [1, ACC_COLS], f32, isOutput=True)

    with ExitStack() as ctx:
        tc = ctx.enter_context(tile.TileContext(nc))
        xpool = ctx.enter_context(tc.tile_pool(name="x", bufs=2))
        tpool = ctx.enter_context(tc.tile_pool(name="t", bufs=2))
        accpool = ctx.enter_context(tc.tile_pool(name="acc", bufs=1))
        pspool = ctx.enter_context(tc.tile_pool(name="ps", bufs=1, space="PSUM"))

        EF = NCLASS * F
        ABN = accpool.tile([P, ACC_COLS], f32)
        nc.vector.memset(ABN[:], 0.0)
        ones = accpool.tile([P, 1], f32)
        nc.vector.memset(ones[:], 1.0)
        # persistent manually double-buffered scratch (avoids Tile pool
        # release-waits, which overflow the 1-sync-wait ISA limit)
        Ebuf = accpool.tile([P, 2 * EF], f32)
        Sbuf = accpool.tile([P, 2 * F], f32)
        Lbuf = accpool.tile([P, 2 * F], f32)
        junk = accpool.tile([P, F], f32)
        pabs = accpool.tile([P, 1], f32)   # DVE absorber dst
        pdve = accpool.tile([P, 1], f32)   # DVE->ACT probe src
        pscr = accpool.tile([P, 1], f32)   # ACT probe dst

        probes = {}
        for k in range(NCHUNK):
            h = k % 2
            X = xpool.tile([P, EF], f32, tag="x")
            xdma = nc.scalar.dma_start(
                X[:].rearrange("p (c f) -> p c f", c=NCLASS),
                logits[:, :, k * F:(k + 1) * F].rearrange("c p f -> p c f"))
            Ti = tpool.tile([P, F], i32, tag="ti")
            tdma = nc.scalar.dma_start(Ti[:], target[:, k * F:(k + 1) * F])
            if k >= 2:
                # Order this chunk's DMAs after the probe that made ACT
                # observe DVE's consumption of the recycled buffers, so the
                # DMACopy needs no extra sync-wait (1-wait ISA limit).
                tile.add_dep_helper(xdma.ins, probes[k - 2], reason="recycle absorb")
                tile.add_dep_helper(tdma.ins, probes[k - 2], reason="recycle absorb")

            E = Ebuf[:, h * EF:(h + 1) * EF]
            for c in range(NCLASS):
                nc.scalar.activation(E[:, c * F:(c + 1) * F], X[:, c * F:(c + 1) * F], AF.Exp)

            S = Sbuf[:, h * F:(h + 1) * F]
            nc.vector.tensor_reduce(
                S[:], E[:].rearrange("p (c f) -> p f c", c=NCLASS),
                axis=mybir.AxisListType.X, op=ALU.add)
            LSE = Lbuf[:, h * F:(h + 1) * F]
            log_ins = nc.scalar.activation(LSE[:], S[:], AF.Ln).ins

            # Drain instructions accept many sync-waits; use one as the
            # absorber for ALL of this chunk's cross-engine edges so every
            # following DVE instruction needs at most its self-wait.
            dr = nc.vector.drain()
            tile.add_dep_helper(dr.ins, xdma.ins, reason="absorb x dma")
            tile.add_dep_helper(dr.ins, tdma.ins, reason="absorb t dma")
            tile.add_dep_helper(dr.ins, log_ins, reason="absorb log")
            for c in range(NCLASS):
                # A_c partial: sum over free of (T==c)*logits_c
                stt = nc.vector.scalar_tensor_tensor(
                    out=junk[:], in0=Ti[:], scalar=float(c), in1=X[:, c * F:(c + 1) * F],
                    op0=ALU.is_equal, op1=ALU.mult,
                    accum_out=ABN[:, 0 * SEC + k * NCLASS + c: 0 * SEC + k * NCLASS + c + 1])
                if c == 0:
                    # force the drain ahead of the whole stt block (ordered
                    # among themselves by the junk WAW chain)
                    tile.add_dep_helper(stt.ins, dr.ins, reason="stt after drain")
            for c in range(NCLASS):
                # B_c partial: sum over free of (T==c)*lse
                nc.vector.scalar_tensor_tensor(
                    out=junk[:], in0=Ti[:], scalar=float(c), in1=LSE[:],
                    op0=ALU.is_equal, op1=ALU.mult,
                    accum_out=ABN[:, 1 * SEC + k * NCLASS + c: 1 * SEC + k * NCLASS + c + 1])
            for c in range(NCLASS):
                # N_c partial: sum over free of (T==c)
                nc.vector.tensor_scalar(
                    out=junk[:], in0=Ti[:], scalar1=float(c), scalar2=1.0,
                    op0=ALU.is_equal, op1=ALU.mult,
                    accum_out=ABN[:, 2 * SEC + k * NCLASS + c: 2 * SEC + k * NCLASS + c + 1])
            nc.vector.tensor_copy(pdve[:], junk[:, 0:1])
            probes[k] = nc.scalar.copy(pscr[:], pdve[:]).ins

        ps = pspool.tile([1, ACC_COLS], f32)
        mm = nc.tensor.matmul(ps[:], lhsT=ones[:], rhs=ABN[:], start=True, stop=True)
        dr2 = nc.scalar.drain()
        tile.add_dep_helper(dr2.ins, mm.ins, reason="absorb matmul")
        res = accpool.tile([1, ACC_COLS], f32)
        nc.scalar.copy(res[:], ps[:])
        nc.scalar.dma_start(out[:, :], res[:])

    return nc


_CACHE = {}


def _get_nc():
    if "nc" not in _CACHE:
        _CACHE["nc"] = _build()
    return _CACHE["nc"]


def _run(logits, target, trace=False):
    nc = _get_nc()
    in_maps = []
    for i in range(NCORES):
        in_maps.append({
            "logits": np.ascontiguousarray(logits[i].reshape(NCLASS, P, COLS)),
            "target": np.ascontiguousarray(target[i].reshape(P, COLS)),
        })
    r = run_bass_kernel_spmd(nc, in_maps, core_ids=list(range(NCORES)), trace=trace)
    return r


def _combine(results):
    A = np.zeros(NCLASS, np.float64)
    B = np.zeros(NCLASS, np.float64)
    N = np.zeros(NCLASS, np.float64)
    for i in range(NCORES):
        r = results[i]["out"].astype(np.float64).reshape(3, NCHUNK, NCLASS).sum(axis=1)
        A += r[0]
        B += r[1]
        N += r[2]
    w = np.where(N > 0, (1.0 - BETA) / (1.0 - BETA ** N), 0.0)
    num = float((w * (A - B)).sum())
    den = float((w * N).sum())
    return np.float32(-num / den)


def kernel(logits, target):
    assert logits.shape == (NCORES, NCLASS, 512, 1024) and logits.dtype == np.float32
    assert target.shape == (NCORES, 512, 1024) and target.dtype == np.int32
    r = _run(logits, target, trace=False)
    return _combine(r.results)



# revision 5
# speedup vs baseline: 1.2579x; 1.0023x over previous
"""Class-balanced softmax cross-entropy loss on 8 Trainium2 NeuronCores.

Math (per the reference nn.Module):
  counts N_c   = histogram of target over classes (whole batch)
  weights w_c  = (1-beta)/(1-beta^N_c), 0 where N_c == 0
  logp         = log_softmax(logits, axis=C)
  loss         = -sum_pix w[t] * logp[t_pix] / sum_pix w[t]

Decomposition used here: per core (data-parallel over batch B=8, one batch
item per core) compute per-class partials
  N_c = sum(target == c)
  A_c = sum_{target==c} logits[c]
  B_c = sum_{target==c} lse          (lse = log(sum_c' exp(logits[c'])))
Then on host: N = sum_cores N_c; w from N;
  loss = -(w . (A - B)) / (w . N)
No collectives needed; each core returns 3*19 floats.
"""

import numpy as np
from contextlib import ExitStack
import sys

for _p in ("/opt/trn_rl_repo",):
    if _p not in sys.path:
        sys.path.insert(0, _p)

from concourse import bass, mybir, tile
from concourse.bass_utils import run_bass_kernel_spmd

NCLASS = 19
BETA = 0.999
NCORES = 8
HW = 512 * 1024          # pixels per batch item (= per core)
P = 128                  # SBUF partitions
COLS = HW // P           # 4096
F = 512                  # free-dim chunk
NCHUNK = COLS // F       # 8

f32 = mybir.dt.float32
i32 = mybir.dt.int32
AF = mybir.ActivationFunctionType
ALU = mybir.AluOpType

# accumulator column layout: [A (NCHUNK*NCLASS) | B (NCLASS) | N (NCLASS)]
SEC = NCHUNK * NCLASS          # 152
ACC_COLS = SEC + 2 * NCLASS    # 190


def _build():
    """Raw-bass pipeline with manual semaphores.

    Engine roles per chunk k (buffer half h=k%2):
      ACT: issue X/T DMAs, exp x19, log; DVE: reduce(sumexp), 57 stt/ts
      accumulations; PE: final partition-reduce matmul.
    Cross-engine edges via explicit wait_ge/then_inc; within-engine order is
    program order. Transitive implications (exp done => X landed) are used
    to keep the wait count low.
    """
    nc = bass.Bass()
    logits = nc.declare_dram_parameter("logits", [NCLASS, P, COLS], f32, isOutput=False)
    target = nc.declare_dram_parameter("target", [P, COLS], i32, isOutput=False)
    out = nc.declare_dram_parameter("out", [1, ACC_COLS], f32, isOutput=True)

    EF = NCLASS * F
    bf16 = mybir.dt.bfloat16
    X2 = nc.alloc_sbuf_tensor("X2", [P, 2 * EF], f32)
    E2 = nc.alloc_sbuf_tensor("E2", [P, 2 * EF], bf16)
    Ti2 = nc.alloc_sbuf_tensor("Ti2", [P, 2 * F], i32)
    TfF = nc.alloc_sbuf_tensor("TfF", [P, COLS], f32)
    LF = nc.alloc_sbuf_tensor("LF", [P, COLS], f32)
    junk = nc.alloc_sbuf_tensor("junk", [P, COLS], f32)
    ABN = nc.alloc_sbuf_tensor("ABN", [P, ACC_COLS], f32)
    ones = nc.alloc_sbuf_tensor("ones", [P, 1], f32)
    ones_f = nc.alloc_sbuf_tensor("ones_f", [P, F], f32)
    res = nc.alloc_sbuf_tensor("res", [1, ACC_COLS], f32)
    ps = nc.alloc_psum_tensor("ps", [1, ACC_COLS], f32)

    with (
        nc.Block() as block,
        nc.semaphore("sem_x") as sem_x,
        nc.semaphore("sem_t") as sem_t,
        nc.semaphore("sem_exp") as sem_exp,
        nc.semaphore("sem_red") as sem_red,
        nc.semaphore("sem_log") as sem_log,
        nc.semaphore("sem_done") as sem_done,
        nc.semaphore("sem_mm") as sem_mm,
        nc.semaphore("sem_res") as sem_res,
        nc.semaphore("sem_out") as sem_out,
    ):
        @block.sync
        def _(sp):
            # all DMA issues live on the otherwise-idle SP queue (issuing
            # from ACT cost ~3.2us of ACT time per dma_start)
            for k in range(NCHUNK):
                h = k % 2
                if k >= 2:
                    sp.wait_ge(sem_done, k - 1)   # DVE done with bufs k-2
                    sp.wait_ge(sem_exp, k - 1)    # exp read X2[h] of k-2
                sp.dma_start(
                    X2[:, h * EF:(h + 1) * EF].rearrange("p (c f) -> p c f", c=NCLASS),
                    logits[:, :, k * F:(k + 1) * F].rearrange("c p f -> p c f"),
                ).then_inc(sem_x, 16)
                sp.dma_start(
                    Ti2[:, h * F:(h + 1) * F], target[:, k * F:(k + 1) * F],
                ).then_inc(sem_t, 16)
            sp.wait_ge(sem_res, 1)
            sp.dma_start(out[:, :], res[:]).then_inc(sem_out, 16)
            sp.wait_ge(sem_out, 16)

        @block.scalar
        def _(act):
            for k in range(NCHUNK):
                h = k % 2
                act.wait_ge(sem_x, 16 * (k + 1))
                act.activation(
                    E2[:, h * EF:(h + 1) * EF],
                    X2[:, h * EF:(h + 1) * EF], AF.Exp,
                ).then_inc(sem_exp, 1)
                act.wait_ge(sem_red, k + 1)
                act.activation(
                    LF[:, k * F:(k + 1) * F], E2[:, h * EF: h * EF + F], AF.Ln,
                ).then_inc(sem_log, 1)
            # tail: psum -> sbuf; SP does the out DMA
            act.wait_ge(sem_mm, 1)
            act.copy(res[:], ps[:]).then_inc(sem_res, 1)

        @block.vector
        def _(dve):
            dve.memset(ABN[:], 0.0)
            dve.memset(ones[:], 1.0)
            dve.memset(ones_f[:], 1.0)
            for k in range(NCHUNK):
                h = k % 2
                dve.wait_ge(sem_exp, k + 1)   # E ready (implies X landed)
                # sumexp via in-place bf16 fold tree (tt-add runs 2x in bf16;
                # the strided tensor_reduce it replaces ran at ~1.7ns/elem)
                E = E2[:, h * EF:(h + 1) * EF]
                dve.tensor_tensor(out=E[:, 0:9 * F], in0=E[:, 0:9 * F],
                                  in1=E[:, 9 * F:18 * F], op=ALU.add)
                dve.tensor_tensor(out=E[:, 0:4 * F], in0=E[:, 0:4 * F],
                                  in1=E[:, 4 * F:8 * F], op=ALU.add)
                dve.tensor_tensor(out=E[:, 0:2 * F], in0=E[:, 0:2 * F],
                                  in1=E[:, 2 * F:4 * F], op=ALU.add)
                dve.tensor_tensor(out=E[:, 0:F], in0=E[:, 0:F],
                                  in1=E[:, F:2 * F], op=ALU.add)
                dve.tensor_tensor(out=E[:, 0:F], in0=E[:, 0:F],
                                  in1=E[:, 8 * F:9 * F], op=ALU.add)
                dve.tensor_tensor(
                    out=E[:, 0:F], in0=E[:, 0:F],
                    in1=E[:, 18 * F:19 * F], op=ALU.add,
                ).then_inc(sem_red, 1)
                dve.wait_ge(sem_t, 16 * (k + 1))
                Ti = TfF[:, k * F:(k + 1) * F]
                dve.tensor_copy(Ti[:], Ti2[:, h * F:(h + 1) * F])
                for c in range(NCLASS):
                    ins = dve.scalar_tensor_tensor(
                        out=junk[:, 0:F], in0=Ti[:], scalar=float(c),
                        in1=X2[:, h * EF + c * F: h * EF + (c + 1) * F],
                        op0=ALU.is_equal, op1=ALU.mult,
                        accum_out=ABN[:, k * NCLASS + c: k * NCLASS + c + 1])
                    if c == NCLASS - 1:
                        ins.then_inc(sem_done, 1)   # X2[h]/Ti2[h] free
            # tail: B and N as single full-width passes (amortizes the
            # ~380ns fixed cost per accumulating op: 38 ops instead of 304)
            dve.wait_ge(sem_log, NCHUNK)
            for c in range(NCLASS):
                dve.scalar_tensor_tensor(
                    out=junk[:], in0=TfF[:], scalar=float(c), in1=LF[:],
                    op0=ALU.is_equal, op1=ALU.mult,
                    accum_out=ABN[:, SEC + c: SEC + c + 1])
            for c in range(NCLASS):
                ins = dve.tensor_scalar(
                    out=junk[:], in0=TfF[:], scalar1=float(c), scalar2=None,
                    op0=ALU.is_equal, op1=ALU.add,
                    accum_out=ABN[:, SEC + NCLASS + c: SEC + NCLASS + c + 1])
                if c == NCLASS - 1:
                    ins.then_inc(sem_done, 1)

        @block.tensor
        def _(pe):
            pe.wait_ge(sem_done, NCHUNK + 1)
            pe.matmul(ps[:], lhsT=ones[:], rhs=ABN[:], start=True, stop=True).then_inc(sem_mm, 1)

    return nc


def _build_tile_unused():
    nc = bass.Bass()
    logits = nc.declare_dram_parameter("logits", [NCLASS, P, COLS], f32, isOutput=False)
    target = nc.declare_dram_parameter("target", [P, COLS], i32, isOutput=False)
    out = nc.declare_dram_parameter("out", [1, ACC_COLS], f32, isOutput=True)

    with ExitStack() as ctx:
        tc = ctx.enter_context(tile.TileContext(nc))
        xpool = ctx.enter_context(tc.tile_pool(name="x", bufs=2))
        tpool = ctx.enter_context(tc.tile_pool(name="t", bufs=2))
        accpool = ctx.enter_context(tc.tile_pool(name="acc", bufs=1))
        pspool = ctx.enter_context(tc.tile_pool(name="ps", bufs=1, space="PSUM"))

        EF = NCLASS * F
        ABN = accpool.tile([P, ACC_COLS], f32)
        nc.vector.memset(ABN[:], 0.0)
        ones = accpool.tile([P, 1], f32)
        nc.vector.memset(ones[:], 1.0)
        # persistent manually double-buffered scratch (avoids Tile pool
        # release-waits, which overflow the 1-sync-wait ISA limit)
        Ebuf = accpool.tile([P, 2 * EF], f32)
        Sbuf = accpool.tile([P, 2 * F], f32)
        Lbuf = accpool.tile([P, 2 * F], f32)
        junk = accpool.tile([P, F], f32)
        pabs = accpool.tile([P, 1], f32)   # DVE absorber dst
        pdve = accpool.tile([P, 1], f32)   # DVE->ACT probe src
        pscr = accpool.tile([P, 1], f32)   # ACT probe dst

        probes = {}
        for k in range(NCHUNK):
            h = k % 2
            X = xpool.tile([P, EF], f32, tag="x")
            xdma = nc.scalar.dma_start(
                X[:].rearrange("p (c f) -> p c f", c=NCLASS),
                logits[:, :, k * F:(k + 1) * F].rearrange("c p f -> p c f"))
            Ti = tpool.tile([P, F], i32, tag="ti")
            tdma = nc.scalar.dma_start(Ti[:], target[:, k * F:(k + 1) * F])
            if k >= 2:
                # Order this chunk's DMAs after the probe that made ACT
                # observe DVE's consumption of the recycled buffers, so the
                # DMACopy needs no extra sync-wait (1-wait ISA limit).
                tile.add_dep_helper(xdma.ins, probes[k - 2], reason="recycle absorb")
                tile.add_dep_helper(tdma.ins, probes[k - 2], reason="recycle absorb")

            E = Ebuf[:, h * EF:(h + 1) * EF]
            for c in range(NCLASS):
                nc.scalar.activation(E[:, c * F:(c + 1) * F], X[:, c * F:(c + 1) * F], AF.Exp)

            S = Sbuf[:, h * F:(h + 1) * F]
            nc.vector.tensor_reduce(
                S[:], E[:].rearrange("p (c f) -> p f c", c=NCLASS),
                axis=mybir.AxisListType.X, op=ALU.add)
            LSE = Lbuf[:, h * F:(h + 1) * F]
            log_ins = nc.scalar.activation(LSE[:], S[:], AF.Ln).ins

            # Drain instructions accept many sync-waits; use one as the
            # absorber for ALL of this chunk's cross-engine edges so every
            # following DVE instruction needs at most its self-wait.
            dr = nc.vector.drain()
            tile.add_dep_helper(dr.ins, xdma.ins, reason="absorb x dma")
            tile.add_dep_helper(dr.ins, tdma.ins, reason="absorb t dma")
            tile.add_dep_helper(dr.ins, log_ins, reason="absorb log")
            for c in range(NCLASS):
                # A_c partial: sum over free of (T==c)*logits_c
                stt = nc.vector.scalar_tensor_tensor(
                    out=junk[:], in0=Ti[:], scalar=float(c), in1=X[:, c * F:(c + 1) * F],
                    op0=ALU.is_equal, op1=ALU.mult,
                    accum_out=ABN[:, 0 * SEC + k * NCLASS + c: 0 * SEC + k * NCLASS + c + 1])
                if c == 0:
                    # force the drain ahead of the whole stt block (ordered
                    # among themselves by the junk WAW chain)
                    tile.add_dep_helper(stt.ins, dr.ins, reason="stt after drain")
            for c in range(NCLASS):
                # B_c partial: sum over free of (T==c)*lse
                nc.vector.scalar_tensor_tensor(
                    out=junk[:], in0=Ti[:], scalar=float(c), in1=LSE[:],
                    op0=ALU.is_equal, op1=ALU.mult,
                    accum_out=ABN[:, 1 * SEC + k * NCLASS + c: 1 * SEC + k * NCLASS + c + 1])
            for c in range(NCLASS):
                # N_c partial: sum over free of (T==c)
                nc.vector.tensor_scalar(
                    out=junk[:], in0=Ti[:], scalar1=float(c), scalar2=1.0,
                    op0=ALU.is_equal, op1=ALU.mult,
                    accum_out=ABN[:, 2 * SEC + k * NCLASS + c: 2 * SEC + k * NCLASS + c + 1])
            nc.vector.tensor_copy(pdve[:], junk[:, 0:1])
            probes[k] = nc.scalar.copy(pscr[:], pdve[:]).ins

        ps = pspool.tile([1, ACC_COLS], f32)
        mm = nc.tensor.matmul(ps[:], lhsT=ones[:], rhs=ABN[:], start=True, stop=True)
        dr2 = nc.scalar.drain()
        tile.add_dep_helper(dr2.ins, mm.ins, reason="absorb matmul")
        res = accpool.tile([1, ACC_COLS], f32)
        nc.scalar.copy(res[:], ps[:])
        nc.scalar.dma_start(out[:, :], res[:])

    return nc


_CACHE = {}


def _get_nc():
    if "nc" not in _CACHE:
        _CACHE["nc"] = _build()
    return _CACHE["nc"]


def _run(logits, target, trace=False):
    nc = _get_nc()
    in_maps = []
    for i in range(NCORES):
        in_maps.append({
            "logits": np.ascontiguousarray(logits[i].reshape(NCLASS, P, COLS)),
            "target": np.ascontiguousarray(target[i].reshape(P, COLS)),
        })
    r = run_bass_kernel_spmd(nc, in_maps, core_ids=list(range(NCORES)), trace=trace)
    return r


def _combine(results):
    A = np.zeros(NCLASS, np.float64)
    B = np.zeros(NCLASS, np.float64)
    N = np.zeros(NCLASS, np.float64)
    for i in range(NCORES):
        r = results[i]["out"].astype(np.float64).reshape(-1)
        A += r[:SEC].reshape(NCHUNK, NCLASS).sum(axis=0)
        B += r[SEC:SEC + NCLASS]
        N += r[SEC + NCLASS:]
    w = np.where(N > 0, (1.0 - BETA) / (1.0 - BETA ** N), 0.0)
    num = float((w * (A - B)).sum())
    den = float((w * N).sum())
    return np.float32(-num / den)


def kernel(logits, target):
    assert logits.shape == (NCORES, NCLASS, 512, 1024) and logits.dtype == np.float32
    assert target.shape == (NCORES, 512, 1024) and target.dtype == np.int32
    r = _run(logits, target, trace=False)
    return _combine(r.results)



# revision 7
# speedup vs baseline: 1.8070x; 1.4365x over previous
"""Class-balanced softmax cross-entropy loss on 8 Trainium2 NeuronCores.

Math (per the reference nn.Module):
  counts N_c   = histogram of target over classes (whole batch)
  weights w_c  = (1-beta)/(1-beta^N_c), 0 where N_c == 0
  logp         = log_softmax(logits, axis=C)
  loss         = -sum_pix w[t] * logp[t_pix] / sum_pix w[t]

Decomposition used here: per core (data-parallel over batch B=8, one batch
item per core) compute per-class partials
  N_c = sum(target == c)
  A_c = sum_{target==c} logits[c]
  B_c = sum_{target==c} lse          (lse = log(sum_c' exp(logits[c'])))
Then on host: N = sum_cores N_c; w from N;
  loss = -(w . (A - B)) / (w . N)
No collectives needed; each core returns 3*19 floats.
"""

import numpy as np
from contextlib import ExitStack
import sys

for _p in ("/opt/trn_rl_repo",):
    if _p not in sys.path:
        sys.path.insert(0, _p)

from concourse import bass, mybir, tile
from concourse.bass_utils import run_bass_kernel_spmd

NCLASS = 19
BETA = 0.999
NCORES = 8
HW = 512 * 1024          # pixels per batch item (= per core)
P = 128                  # SBUF partitions
COLS = HW // P           # 4096
F = 512                  # free-dim chunk
NCHUNK = COLS // F       # 8

f32 = mybir.dt.float32
i32 = mybir.dt.int32
AF = mybir.ActivationFunctionType
ALU = mybir.AluOpType

# accumulator column layout: [A (NCHUNK*NCLASS) | B (NCLASS) | N (NCLASS)]
SEC = NCHUNK * NCLASS          # 152
ACC_COLS = SEC + 2 * NCLASS    # 190


def _build():
    """Raw-bass pipeline with manual semaphores.

    Engine roles per chunk k (buffer half h=k%2):
      ACT: issue X/T DMAs, exp x19, log; DVE: reduce(sumexp), 57 stt/ts
      accumulations; PE: final partition-reduce matmul.
    Cross-engine edges via explicit wait_ge/then_inc; within-engine order is
    program order. Transitive implications (exp done => X landed) are used
    to keep the wait count low.
    """
    nc = bass.Bass()
    logits = nc.declare_dram_parameter("logits", [NCLASS, P, COLS], f32, isOutput=False)
    target = nc.declare_dram_parameter("target", [P, COLS], i32, isOutput=False)
    out = nc.declare_dram_parameter("out", [1, ACC_COLS], f32, isOutput=True)

    EF = NCLASS * F
    bf16 = mybir.dt.bfloat16
    X2 = nc.alloc_sbuf_tensor("X2", [P, 2 * EF], f32)
    E2 = nc.alloc_sbuf_tensor("E2", [P, 2 * EF], bf16)
    Ti2 = nc.alloc_sbuf_tensor("Ti2", [P, 2 * F], i32)
    TfF = nc.alloc_sbuf_tensor("TfF", [P, COLS], f32)
    LF = nc.alloc_sbuf_tensor("LF", [P, COLS], f32)
    junk = nc.alloc_sbuf_tensor("junk", [P, COLS], f32)
    junka = nc.alloc_sbuf_tensor("junka", [P, COLS], bf16)
    BIAS = nc.alloc_sbuf_tensor("BIAS", [P, NCLASS], f32)
    ABN = nc.alloc_sbuf_tensor("ABN", [P, ACC_COLS], f32)
    ones = nc.alloc_sbuf_tensor("ones", [P, 1], f32)
    ones_f = nc.alloc_sbuf_tensor("ones_f", [P, F], f32)
    res = nc.alloc_sbuf_tensor("res", [1, ACC_COLS], f32)
    ps = nc.alloc_psum_tensor("ps", [1, ACC_COLS], f32)

    with (
        nc.Block() as block,
        nc.semaphore("sem_x") as sem_x,
        nc.semaphore("sem_t") as sem_t,
        nc.semaphore("sem_exp") as sem_exp,
        nc.semaphore("sem_red") as sem_red,
        nc.semaphore("sem_log") as sem_log,
        nc.semaphore("sem_done") as sem_done,
        nc.semaphore("sem_mm") as sem_mm,
        nc.semaphore("sem_res") as sem_res,
        nc.semaphore("sem_nact") as sem_nact,
        nc.semaphore("sem_out") as sem_out,
    ):
        @block.sync
        def _(sp):
            # all DMA issues live on the otherwise-idle SP queue (issuing
            # from ACT cost ~3.2us of ACT time per dma_start)
            for k in range(NCHUNK):
                h = k % 2
                if k >= 2:
                    sp.wait_ge(sem_done, k - 1)   # DVE done with bufs k-2
                    sp.wait_ge(sem_exp, k - 1)    # exp read X2[h] of k-2
                sp.dma_start(
                    X2[:, h * EF:(h + 1) * EF].rearrange("p (c f) -> p c f", c=NCLASS),
                    logits[:, :, k * F:(k + 1) * F].rearrange("c p f -> p c f"),
                ).then_inc(sem_x, 16)
                sp.dma_start(
                    Ti2[:, h * F:(h + 1) * F], target[:, k * F:(k + 1) * F],
                ).then_inc(sem_t, 16)
            sp.wait_ge(sem_res, 1)
            sp.dma_start(out[:, :], res[:]).then_inc(sem_out, 16)
            sp.wait_ge(sem_out, 16)

        @block.scalar
        def _(act):
            for k in range(NCHUNK):
                h = k % 2
                act.wait_ge(sem_x, 16 * (k + 1))
                act.activation(
                    E2[:, h * EF:(h + 1) * EF],
                    X2[:, h * EF:(h + 1) * EF], AF.Exp,
                ).then_inc(sem_exp, 1)
                act.wait_ge(sem_red, k + 1)
                act.activation(
                    LF[:, k * F:(k + 1) * F], E2[:, h * EF: h * EF + F], AF.Ln,
                ).then_inc(sem_log, 1)
            # N-family on ACT: D_c = sum relu(t - c + 0.5); counts are
            # recovered on the host by telescoping (exact: half-integers)
            act.wait_ge(sem_done, NCHUNK)   # all TfF slices written
            for c in range(NCLASS):
                ins = act.activation(
                    junka[:], TfF[:], AF.Relu, bias=BIAS[:, c:c + 1],
                    accum_out=ABN[:, SEC + NCLASS + c: SEC + NCLASS + c + 1])
                if c == NCLASS - 1:
                    ins.then_inc(sem_nact, 1)
            # tail: psum -> sbuf; SP does the out DMA
            act.wait_ge(sem_mm, 1)
            act.copy(res[:], ps[:]).then_inc(sem_res, 1)

        @block.vector
        def _(dve):
            dve.memset(ABN[:], 0.0)
            dve.memset(ones[:], 1.0)
            dve.memset(ones_f[:], 1.0)
            for c in range(NCLASS):
                dve.memset(BIAS[:, c:c + 1], 0.5 - float(c))
            for k in range(NCHUNK):
                h = k % 2
                # A-stt first: needs only X + t, so no stall on ACT's exp
                dve.wait_ge(sem_t, 16 * (k + 1))
                Ti = TfF[:, k * F:(k + 1) * F]
                dve.tensor_copy(Ti[:], Ti2[:, h * F:(h + 1) * F])
                dve.wait_ge(sem_x, 16 * (k + 1))
                for c in range(NCLASS):
                    ins = dve.scalar_tensor_tensor(
                        out=junk[:, 0:F], in0=Ti[:], scalar=float(c),
                        in1=X2[:, h * EF + c * F: h * EF + (c + 1) * F],
                        op0=ALU.is_equal, op1=ALU.mult,
                        accum_out=ABN[:, k * NCLASS + c: k * NCLASS + c + 1])
                    if c == NCLASS - 1:
                        ins.then_inc(sem_done, 1)   # X2[h]/Ti2[h] free
                dve.wait_ge(sem_exp, k + 1)   # E ready
                # sumexp via in-place bf16 fold tree (tt-add runs 2x in bf16)
                E = E2[:, h * EF:(h + 1) * EF]
                dve.tensor_tensor(out=E[:, 0:9 * F], in0=E[:, 0:9 * F],
                                  in1=E[:, 9 * F:18 * F], op=ALU.add)
                dve.tensor_tensor(out=E[:, 0:4 * F], in0=E[:, 0:4 * F],
                                  in1=E[:, 4 * F:8 * F], op=ALU.add)
                dve.tensor_tensor(out=E[:, 0:2 * F], in0=E[:, 0:2 * F],
                                  in1=E[:, 2 * F:4 * F], op=ALU.add)
                dve.tensor_tensor(out=E[:, 0:F], in0=E[:, 0:F],
                                  in1=E[:, F:2 * F], op=ALU.add)
                dve.tensor_tensor(out=E[:, 0:F], in0=E[:, 0:F],
                                  in1=E[:, 8 * F:9 * F], op=ALU.add)
                dve.tensor_tensor(
                    out=E[:, 0:F], in0=E[:, 0:F],
                    in1=E[:, 18 * F:19 * F], op=ALU.add,
                ).then_inc(sem_red, 1)
            # tail: B as a single full-width pass
            dve.wait_ge(sem_log, NCHUNK)
            for c in range(NCLASS):
                ins = dve.scalar_tensor_tensor(
                    out=junk[:], in0=TfF[:], scalar=float(c), in1=LF[:],
                    op0=ALU.is_equal, op1=ALU.mult,
                    accum_out=ABN[:, SEC + c: SEC + c + 1])
                if c == NCLASS - 1:
                    ins.then_inc(sem_done, 1)

        @block.tensor
        def _(pe):
            pe.wait_ge(sem_done, NCHUNK + 1)
            pe.wait_ge(sem_nact, 1)
            pe.matmul(ps[:], lhsT=ones[:], rhs=ABN[:], start=True, stop=True).then_inc(sem_mm, 1)

    return nc


def _build_tile_unused():
    nc = bass.Bass()
    logits = nc.declare_dram_parameter("logits", [NCLASS, P, COLS], f32, isOutput=False)
    target = nc.declare_dram_parameter("target", [P, COLS], i32, isOutput=False)
    out = nc.declare_dram_parameter("out", [1, ACC_COLS], f32, isOutput=True)

    with ExitStack() as ctx:
        tc = ctx.enter_context(tile.TileContext(nc))
        xpool = ctx.enter_context(tc.tile_pool(name="x", bufs=2))
        tpool = ctx.enter_context(tc.tile_pool(name="t", bufs=2))
        accpool = ctx.enter_context(tc.tile_pool(name="acc", bufs=1))
        pspool = ctx.enter_context(tc.tile_pool(name="ps", bufs=1, space="PSUM"))

        EF = NCLASS * F
        ABN = accpool.tile([P, ACC_COLS], f32)
        nc.vector.memset(ABN[:], 0.0)
        ones = accpool.tile([P, 1], f32)
        nc.vector.memset(ones[:], 1.0)
        # persistent manually double-buffered scratch (avoids Tile pool
        # release-waits, which overflow the 1-sync-wait ISA limit)
        Ebuf = accpool.tile([P, 2 * EF], f32)
        Sbuf = accpool.tile([P, 2 * F], f32)
        Lbuf = accpool.tile([P, 2 * F], f32)
        junk = accpool.tile([P, F], f32)
        pabs = accpool.tile([P, 1], f32)   # DVE absorber dst
        pdve = accpool.tile([P, 1], f32)   # DVE->ACT probe src
        pscr = accpool.tile([P, 1], f32)   # ACT probe dst

        probes = {}
        for k in range(NCHUNK):
            h = k % 2
            X = xpool.tile([P, EF], f32, tag="x")
            xdma = nc.scalar.dma_start(
                X[:].rearrange("p (c f) -> p c f", c=NCLASS),
                logits[:, :, k * F:(k + 1) * F].rearrange("c p f -> p c f"))
            Ti = tpool.tile([P, F], i32, tag="ti")
            tdma = nc.scalar.dma_start(Ti[:], target[:, k * F:(k + 1) * F])
            if k >= 2:
                # Order this chunk's DMAs after the probe that made ACT
                # observe DVE's consumption of the recycled buffers, so the
                # DMACopy needs no extra sync-wait (1-wait ISA limit).
                tile.add_dep_helper(xdma.ins, probes[k - 2], reason="recycle absorb")
                tile.add_dep_helper(tdma.ins, probes[k - 2], reason="recycle absorb")

            E = Ebuf[:, h * EF:(h + 1) * EF]
            for c in range(NCLASS):
                nc.scalar.activation(E[:, c * F:(c + 1) * F], X[:, c * F:(c + 1) * F], AF.Exp)

            S = Sbuf[:, h * F:(h + 1) * F]
            nc.vector.tensor_reduce(
                S[:], E[:].rearrange("p (c f) -> p f c", c=NCLASS),
                axis=mybir.AxisListType.X, op=ALU.add)
            LSE = Lbuf[:, h * F:(h + 1) * F]
            log_ins = nc.scalar.activation(LSE[:], S[:], AF.Ln).ins

            # Drain instructions accept many sync-waits; use one as the
            # absorber for ALL of this chunk's cross-engine edges so every
            # following DVE instruction needs at most its self-wait.
            dr = nc.vector.drain()
            tile.add_dep_helper(dr.ins, xdma.ins, reason="absorb x dma")
            tile.add_dep_helper(dr.ins, tdma.ins, reason="absorb t dma")
            tile.add_dep_helper(dr.ins, log_ins, reason="absorb log")
            for c in range(NCLASS):
                # A_c partial: sum over free of (T==c)*logits_c
                stt = nc.vector.scalar_tensor_tensor(
                    out=junk[:], in0=Ti[:], scalar=float(c), in1=X[:, c * F:(c + 1) * F],
                    op0=ALU.is_equal, op1=ALU.mult,
                    accum_out=ABN[:, 0 * SEC + k * NCLASS + c: 0 * SEC + k * NCLASS + c + 1])
                if c == 0:
                    # force the drain ahead of the whole stt block (ordered
                    # among themselves by the junk WAW chain)
                    tile.add_dep_helper(stt.ins, dr.ins, reason="stt after drain")
            for c in range(NCLASS):
                # B_c partial: sum over free of (T==c)*lse
                nc.vector.scalar_tensor_tensor(
                    out=junk[:], in0=Ti[:], scalar=float(c), in1=LSE[:],
                    op0=ALU.is_equal, op1=ALU.mult,
                    accum_out=ABN[:, 1 * SEC + k * NCLASS + c: 1 * SEC + k * NCLASS + c + 1])
            for c in range(NCLASS):
                # N_c partial: sum over free of (T==c)
                nc.vector.tensor_scalar(
                    out=junk[:], in0=Ti[:], scalar1=float(c), scalar2=1.0,
                    op0=ALU.is_equal, op1=ALU.mult,
                    accum_out=ABN[:, 2 * SEC + k * NCLASS + c: 2 * SEC + k * NCLASS + c + 1])
            nc.vector.tensor_copy(pdve[:], junk[:, 0:1])
            probes[k] = nc.scalar.copy(pscr[:], pdve[:]).ins

        ps = pspool.tile([1, ACC_COLS], f32)
        mm = nc.tensor.matmul(ps[:], lhsT=ones[:], rhs=ABN[:], start=True, stop=True)
        dr2 = nc.scalar.drain()
        tile.add_dep_helper(dr2.ins, mm.ins, reason="absorb matmul")
        res = accpool.tile([1, ACC_COLS], f32)
        nc.scalar.copy(res[:], ps[:])
        nc.scalar.dma_start(out[:, :], res[:])

    return nc


_CACHE = {}


def _get_nc():
    if "nc" not in _CACHE:
        _CACHE["nc"] = _build()
    return _CACHE["nc"]


def _run(logits, target, trace=False):
    nc = _get_nc()
    in_maps = []
    for i in range(NCORES):
        in_maps.append({
            "logits": np.ascontiguousarray(logits[i].reshape(NCLASS, P, COLS)),
            "target": np.ascontiguousarray(target[i].reshape(P, COLS)),
        })
    r = run_bass_kernel_spmd(nc, in_maps, core_ids=list(range(NCORES)), trace=trace)
    return r


def _combine(results):
    A = np.zeros(NCLASS, np.float64)
    B = np.zeros(NCLASS, np.float64)
    N = np.zeros(NCLASS, np.float64)
    for i in range(NCORES):
        r = results[i]["out"].astype(np.float64).reshape(-1)
        A += r[:SEC].reshape(NCHUNK, NCLASS).sum(axis=0)
        B += r[SEC:SEC + NCLASS]
        N += r[SEC + NCLASS:]   # D_c = sum relu(t - c + 0.5); telescoped below
    D = np.concatenate([N, [0.0]])
    N = np.zeros(NCLASS, np.float64)
    cnt_ge = 0.0
    for c in range(NCLASS - 1, -1, -1):
        N[c] = 2.0 * (D[c] - D[c + 1] - cnt_ge)
        cnt_ge += N[c]
    w = np.where(N > 0, (1.0 - BETA) / (1.0 - BETA ** N), 0.0)
    num = float((w * (A - B)).sum())
    den = float((w * N).sum())
    return np.float32(-num / den)


def kernel(logits, target):
    assert logits.shape == (NCORES, NCLASS, 512, 1024) and logits.dtype == np.float32
    assert target.shape == (NCORES, 512, 1024) and target.dtype == np.int32
    r = _run(logits, target, trace=False)
    return _combine(r.results)

